# revision 25
# baseline (speedup 1.0000x reference)
"""Multi-headed causal attention (B=2, S=2048, D=1024, H=16, DK=DV=64) on 8
Trainium2 NeuronCores.

Sharding: HEAD-parallel attention + QUERY-parallel output projection.
Core c owns heads {2c, 2c+1} for BOTH batches. It projects K/Q/V only for
its two heads (zero redundant FLOPs), runs the full causal attention for
them, then a single 1MB AllToAll redistributes the normalized attention
outputs so core c ends up with all 16 heads for query chunk
(batch c//4, rows 512*(c%4) ...). Each core then output-projects its own
512 queries. Head-sharding makes the fine-grained causal tile structure
(only kt <= t score tiles, 34 banks of 4 tiles per head-batch) IDENTICAL
on every core, which a query-sharded SPMD program cannot do.

All matmul operands are bf16 (full PE rate at any free size, half the DMA
bytes, and far less PE power than fp32r -> avoids the 50% power throttle
the fp32r baseline hit). PSUM accumulation stays f32. Softmax skips
max-subtraction (scores are O(1)); denominators come from an all-ones
column appended to V (an extra output partition, free on the PE); the
reciprocal is one fast-approx DVE op on the [1,512] denominator row,
replicated across partitions by a K=1 matmul. Causal masking multiplies
the 128x128 triangular mask only on diagonal tiles (the padding mask is
all ones in this problem; a general fallback masks every bank).
"""

import numpy as np

B, S, D, H, DK = 2, 2048, 1024, 16, 64
NCORES = 8
NT = S // 128  # 16 tiles per batch
NBANKS = 34    # 136 causal (t,kt) tiles / 4 slots per PSUM bank

_BUILT = {}


def _build_nc(general_mask):
    import concourse.bacc as bacc
    import concourse.mybir as mybir
    from concourse import tile
    from contextlib import ExitStack

    f32 = mybir.dt.float32
    f32r = mybir.dt.float32r
    bf16 = mybir.dt.bfloat16
    AF = mybir.ActivationFunctionType
    ALU = mybir.AluOpType

    nc = bacc.Bacc("TRN2", target_bir_lowering=False, debug=False,
                   num_devices=NCORES)

    # x tensors are [b*1024 + dim, seq] transposed inputs, same on all cores
    xk_t = nc.declare_dram_parameter("xk_t", [2 * D, S], bf16, isOutput=False)
    xq_t = nc.declare_dram_parameter("xq_t", [2 * D, S], bf16, isOutput=False)
    xv_t = nc.declare_dram_parameter("xv_t", [2 * D, S], bf16, isOutput=False)
    # per-core head-pair weight slices
    wk_h = nc.declare_dram_parameter("wk_h", [D, 128], bf16, isOutput=False)
    wq_h = nc.declare_dram_parameter("wq_h", [D, 128], bf16, isOutput=False)
    wv_p = nc.declare_dram_parameter("wv_p", [D, 130], bf16, isOutput=False)
    wo_r = nc.declare_dram_parameter("wo_r", [D, D], bf16, isOutput=False)
    bk_h = nc.declare_dram_parameter("bk_h", [128, 1], f32, isOutput=False)
    bq_h = nc.declare_dram_parameter("bq_h", [128, 1], f32, isOutput=False)
    bv_p = nc.declare_dram_parameter("bv_p", [1, 130], f32, isOutput=False)
    bo_r = nc.declare_dram_parameter("bo_r", [1, D], f32, isOutput=False)
    trimask = nc.declare_dram_parameter("trimask", [128, 128], bf16,
                                        isOutput=False)
    if general_mask:
        maskb = nc.declare_dram_parameter(
            "maskb", [2 * NBANKS * 128, 512], bf16, isOutput=False)
    out = nc.declare_dram_parameter("out", [512, D], f32, isOutput=True)

    with tile.TileContext(nc) as tc:
        with ExitStack() as ctx:
            persist = ctx.enter_context(tc.tile_pool(name="persist", bufs=1))

            # ---- persistent tiles ----
            wk_sb = [persist.tile([128, 128], bf16, name=f"wk{i}",
                                  tag=f"wk{i}") for i in range(8)]
            wq_sb = [persist.tile([128, 128], bf16, name=f"wq{i}",
                                  tag=f"wq{i}") for i in range(8)]
            wv_sb = [persist.tile([128, 130], bf16, name=f"wv{i}",
                                  tag=f"wv{i}") for i in range(8)]
            wo_sb = [persist.tile([128, D], bf16, name=f"wo{i}",
                                  tag=f"wo{i}") for i in range(8)]
            bk_sb = persist.tile([128, 1], f32, name="bk", tag="bk")
            bq_sb = persist.tile([128, 1], f32, name="bq", tag="bq")
            tri_sb = persist.tile([128, 128], bf16, name="tri", tag="tri")
            bvr_sb = persist.tile([1, 130], f32, name="bvr", tag="bvr")
            bor_sb = persist.tile([1, D], f32, name="bor", tag="bor")
            bv_rep = persist.tile([128, 130], f32, name="bvrep", tag="bvrep")
            bo_rep = persist.tile([128, D], f32, name="borep", tag="borep")
            kT = [persist.tile([128, S], bf16, name=f"kT{b}", tag=f"kT{b}")
                  for b in range(B)]
            qT = [persist.tile([128, S], bf16, name=f"qT{b}", tag=f"qT{b}")
                  for b in range(B)]
            v_sb = [[persist.tile([128, 130], bf16, name=f"v{b}_{st}",
                                  tag=f"v{b}_{st}") for st in range(NT)]
                    for b in range(B)]
            navTh = [[persist.tile([64, S], bf16, name=f"nav{b}_{hh}",
                                   tag=f"nav{b}_{hh}") for hh in range(2)]
                     for b in range(B)]
            nall = [persist.tile([128, 512], bf16, name=f"na{i}",
                                 tag=f"na{i}") for i in range(8)]

            # ---- working pools ----
            xs = ctx.enter_context(tc.tile_pool(name="xs", bufs=2))
            amp = ctx.enter_context(tc.tile_pool(name="amp", bufs=3))
            nrm = ctx.enter_context(tc.tile_pool(name="nrm", bufs=2))
            reps = ctx.enter_context(tc.tile_pool(name="reps", bufs=2))
            fop = ctx.enter_context(tc.tile_pool(name="fop", bufs=2))
            pp = ctx.enter_context(tc.tile_pool(name="pp", bufs=2,
                                                space="PSUM"))
            scp = ctx.enter_context(tc.tile_pool(name="scp", bufs=4,
                                                 space="PSUM"))
            avp = ctx.enter_context(tc.tile_pool(name="avp", bufs=2,
                                                 space="PSUM"))
            if general_mask:
                mbp = ctx.enter_context(tc.tile_pool(name="mbp", bufs=4))
            dram = ctx.enter_context(tc.tile_pool(name="dram", bufs=1,
                                                  space="DRAM"))
            a2a_in = [dram.tile([512, 512], bf16, name=f"a2a_in{h}",
                                tag=f"a2a_in{h}") for h in range(2)]
            a2a_out = [dram.tile([512, 512], bf16, name=f"a2a_out{h}",
                                 tag=f"a2a_out{h}") for h in range(2)]

            # ---- P0: K-path loads first so the PE starts ASAP; the rest
            # spread across issue queues and overlap the K projection ----
            for i in range(8):
                nc.sync.dma_start(wk_sb[i][:], wk_h[128 * i:128 * (i + 1), :])
            nc.sync.dma_start(bk_sb[:], bk_h[:])

            # ---- projection helpers ----
            def load_x(param, b):
                tiles = [xs.tile([128, S], bf16, name=f"x{kp}", tag=f"x{kp}")
                         for kp in range(8)]
                for kp in range(8):
                    nc.sync.dma_start(
                        tiles[kp][:],
                        param[D * b + 128 * kp:D * b + 128 * (kp + 1), :])
                return tiles

            def load_x_halves(param, b):
                # two half-loads per tile on alternating queues so the first
                # projection unit starts after ~2MB instead of 4MB
                tiles = [xs.tile([128, S], bf16, name=f"x{kp}", tag=f"x{kp}")
                         for kp in range(8)]
                for h in range(2):
                    for kp in range(8):
                        eng = nc.sync if kp % 2 == 0 else nc.scalar
                        eng.dma_start(
                            tiles[kp][:, 1024 * h:1024 * (h + 1)],
                            param[D * b + 128 * kp:D * b + 128 * (kp + 1),
                                  1024 * h:1024 * (h + 1)])
                return tiles

            def proj_kq_unit(x, w_sb, bias_sb, dst, sc):
                ps = pp.tile([128, 512], f32, name="pp", tag="pp")
                for kp in range(8):
                    nc.tensor.matmul(ps[:], w_sb[kp][:],
                                     x[kp][:, 512 * sc:512 * (sc + 1)],
                                     start=(kp == 0), stop=(kp == 7))
                nc.vector.tensor_scalar_add(
                    dst[:, 512 * sc:512 * (sc + 1)], ps[:], bias_sb[:])

            def proj_v_unit(x, b, st):
                ps = pp.tile([128, 512], f32, name="pp", tag="pp")
                for kp in range(8):
                    nc.tensor.matmul(ps[:, 0:130],
                                     x[kp][:, 128 * st:128 * (st + 1)],
                                     wv_sb[kp][:],
                                     start=(kp == 0), stop=(kp == 7))
                nc.vector.tensor_tensor(v_sb[b][st][:], ps[:, 0:130],
                                        bv_rep[:], ALU.add)

            def b1_proj_gen():
                x = load_x(xk_t, 1)
                yield
                for sc in range(4):
                    proj_kq_unit(x, wk_sb, bk_sb, kT[1], sc)
                    yield
                x = load_x(xq_t, 1)
                yield
                for sc in range(4):
                    proj_kq_unit(x, wq_sb, bq_sb, qT[1], sc)
                    yield
                x = load_x(xv_t, 1)
                yield
                for st in range(NT):
                    proj_v_unit(x, 1, st)
                    yield

            # ---- attention ----
            stream = [(t, kt) for t in range(NT) for kt in range(t + 1)]
            banks = [stream[i:i + 4] for i in range(0, len(stream), 4)]

            def norm_block(b, hh, av, T):
                # denominator row lives on PSUM partition 64; only ACT can
                # shift partitions, DVE lanes are partition-locked
                dg0 = nrm.tile([1, 512], f32, name="dg0", tag="dg0")
                nc.scalar.copy(dg0[:], av[64:65, :])
                dg = nrm.tile([1, 512], f32, name="dg", tag="dg")
                nc.vector.reciprocal_approx_fast(dg[:], dg0[:])
                rep = reps.tile([64, 512], f32, name="rep", tag="rep")
                nc.gpsimd.partition_broadcast(rep[:], dg[:])
                nc.vector.tensor_tensor(
                    navTh[b][hh][:, 512 * T:512 * (T + 1)],
                    av[0:64, :], rep[:], ALU.mult)
                j = 4 * b + T
                nc.sync.dma_start(a2a_in[hh][64 * j:64 * (j + 1), :],
                                  navTh[b][hh][:, 512 * T:512 * (T + 1)])

            def attention(b, hh, filler=None, fill_every=1):
                r0 = 64 * hh
                av = None
                for bi, bank in enumerate(banks):
                    if filler is not None and bi % fill_every == 0:
                        next(filler, None)
                    sc = scp.tile([128, 512], f32, name="sc", tag="sc")
                    for s, (t, kt) in enumerate(bank):
                        nc.tensor.matmul(
                            sc[:, 128 * s:128 * (s + 1)],
                            kT[b][r0:r0 + 64, 128 * kt:128 * (kt + 1)],
                            qT[b][r0:r0 + 64, 128 * t:128 * (t + 1)],
                            start=True, stop=True)
                    am = amp.tile([128, 512], bf16, name="am", tag="am")
                    nc.scalar.activation(am[:], sc[:], AF.Exp, scale=0.125)
                    if general_mask:
                        mb = mbp.tile([128, 512], bf16, name="mb", tag="mb")
                        r = (b * NBANKS + bi) * 128
                        nc.sync.dma_start(mb[:], maskb[r:r + 128, :])
                        nc.vector.tensor_tensor(am[:], am[:], mb[:], ALU.mult)
                    else:
                        for s, (t, kt) in enumerate(bank):
                            if t == kt:
                                nc.vector.tensor_tensor(
                                    am[:, 128 * s:128 * (s + 1)],
                                    am[:, 128 * s:128 * (s + 1)],
                                    tri_sb[:], ALU.mult)
                    for s, (t, kt) in enumerate(bank):
                        if kt == 0 and t % 4 == 0:
                            av = avp.tile([65, 512], f32, name="av", tag="av")
                        nc.tensor.matmul(
                            av[:, 128 * (t % 4):128 * (t % 4 + 1)],
                            v_sb[b][kt][:, 65 * hh:65 * (hh + 1)],
                            am[:, 128 * s:128 * (s + 1)],
                            start=(kt == 0), stop=(kt == t))
                        if kt == t and t % 4 == 3:
                            norm_block(b, hh, av, t // 4)

            # partial (hh=0 heads + bias) output-projection accumulators
            part = [[persist.tile([128, 512], f32, name=f"part{qc}_{oc}",
                                  tag=f"part{qc}_{oc}") for oc in range(2)]
                    for qc in range(4)]

            def outproj_h0_gen():
                # nall0 loads wait on A2A#1 completion (sync queue has
                # nothing urgent then); matmuls are interleaved into
                # attention(1,1), by which time A2A#1 has long finished
                for i in range(8):
                    nc.sync.dma_start(nall[i][0:64, :],
                                      a2a_out[0][64 * i:64 * (i + 1), :])
                yield
                for qc in range(4):
                    for oc in range(2):
                        ps = scp.tile([128, 512], f32, name="sc", tag="sc")
                        for i in range(8):
                            nc.tensor.matmul(
                                ps[:], nall[i][0:64, 128 * qc:128 * (qc + 1)],
                                wo_sb[i][0:64, 512 * oc:512 * (oc + 1)],
                                start=(i == 0), stop=(i == 7))
                        nc.vector.tensor_tensor(
                            part[qc][oc][:], ps[:],
                            bo_rep[:, 512 * oc:512 * (oc + 1)], ALU.add)
                        yield

            # ---- emission schedule ----
            # K-path loads were emitted above; remaining weight loads go on
            # other queues and overlap the K projection
            xk0 = load_x_halves(xk_t, 0)
            for i in range(8):
                nc.scalar.dma_start(wq_sb[i][:],
                                    wq_h[128 * i:128 * (i + 1), :])
                nc.gpsimd.dma_start(wv_sb[i][:],
                                    wv_p[128 * i:128 * (i + 1), :])
                nc.gpsimd.dma_start(wo_sb[i][:],
                                    wo_r[128 * i:128 * (i + 1), :])
            nc.gpsimd.dma_start(bq_sb[:], bq_h[:])
            nc.gpsimd.dma_start(tri_sb[:], trimask[:])
            nc.gpsimd.dma_start(bvr_sb[:], bv_p[:])
            nc.gpsimd.dma_start(bor_sb[:], bo_r[:])
            nc.gpsimd.partition_broadcast(bv_rep[:], bvr_sb[:])
            nc.gpsimd.partition_broadcast(bo_rep[:], bor_sb[:])
            for sc in range(4):
                proj_kq_unit(xk0, wk_sb, bk_sb, kT[0], sc)
            xq0 = load_x_halves(xq_t, 0)
            for sc in range(4):
                proj_kq_unit(xq0, wq_sb, bq_sb, qT[0], sc)
            xv0 = load_x_halves(xv_t, 0)
            for st in range(NT):
                proj_v_unit(xv0, 0, st)

            filler = b1_proj_gen()
            attention(0, 0, filler)
            for _ in filler:  # drain leftovers: b1 proj complete
                pass
            attention(1, 0)
            # hh=0 of both batches done -> first half AllToAll overlaps the
            # remaining attention
            nc.gpsimd.collective_compute(
                "AllToAll", ALU.bypass,
                replica_groups=[list(range(NCORES))],
                ins=[a2a_in[0].opt()], outs=[a2a_out[0].opt()])
            attention(0, 1)
            attention(1, 1, outproj_h0_gen(), fill_every=4)
            nc.gpsimd.collective_compute(
                "AllToAll", ALU.bypass,
                replica_groups=[list(range(NCORES))],
                ins=[a2a_in[1].opt()], outs=[a2a_out[1].opt()])
            for i in range(8):
                nc.sync.dma_start(nall[i][64:128, :],
                                  a2a_out[1][64 * i:64 * (i + 1), :])

            # ---- hh=1 half of the output projection + writeback ----
            for qc in range(4):
                for oc in range(2):
                    ps = scp.tile([128, 512], f32, name="sc", tag="sc")
                    for i in range(8):
                        nc.tensor.matmul(
                            ps[:],
                            nall[i][64:128, 128 * qc:128 * (qc + 1)],
                            wo_sb[i][64:128, 512 * oc:512 * (oc + 1)],
                            start=(i == 0), stop=(i == 7))
                    fo = fop.tile([128, 512], f32, name="fo", tag="fo")
                    nc.vector.tensor_tensor(
                        fo[:], ps[:], part[qc][oc][:], ALU.add)
                    nc.sync.dma_start(
                        out[128 * qc:128 * (qc + 1),
                            512 * oc:512 * (oc + 1)], fo[:])

    nc.compile()
    return nc


def kernel(V, K, Q, padding_mask, Wv_w, Wv_b, Wk_w, Wk_b, Wq_w, Wq_b,
           Wo_w, Wo_b):
    import ml_dtypes
    from concourse.bass_utils import run_bass_kernel_spmd
    bf = ml_dtypes.bfloat16

    V = np.asarray(V, np.float32)
    K = np.asarray(K, np.float32)
    Q = np.asarray(Q, np.float32)
    pm = np.asarray(padding_mask)
    Wv_w = np.asarray(Wv_w, np.float32)
    Wv_b = np.asarray(Wv_b, np.float32)
    Wk_w = np.asarray(Wk_w, np.float32)
    Wk_b = np.asarray(Wk_b, np.float32)
    Wq_w = np.asarray(Wq_w, np.float32)
    Wq_b = np.asarray(Wq_b, np.float32)
    Wo_w = np.asarray(Wo_w, np.float32)
    Wo_b = np.asarray(Wo_b, np.float32)

    general = not bool((pm != 0).all())
    key = "gen" if general else "fast"
    if key not in _BUILT:
        _BUILT[key] = _build_nc(general)
    nc = _BUILT[key]

    xk = np.concatenate(
        [np.ascontiguousarray(K[b].T) for b in range(B)], 0).astype(bf)
    xq = np.concatenate(
        [np.ascontiguousarray(Q[b].T) for b in range(B)], 0).astype(bf)
    xv = np.concatenate(
        [np.ascontiguousarray(V[b].T) for b in range(B)], 0).astype(bf)
    wo_r = np.ascontiguousarray(Wo_w.T).astype(bf)
    bo = Wo_b.reshape(1, D).astype(np.float32)
    tri = (np.arange(128)[:, None] <= np.arange(128)[None, :])

    maskb_arr = None
    if general:
        stream = [(t, kt) for t in range(NT) for kt in range(t + 1)]
        bank_list = [stream[i:i + 4] for i in range(0, len(stream), 4)]
        maskb_arr = np.zeros((2 * NBANKS * 128, 512), np.float32)
        for b in range(B):
            keymask = (pm[b] != 0).astype(np.float32)
            for bi, bank in enumerate(bank_list):
                blk = np.zeros((128, 512), np.float32)
                for s, (t, kt) in enumerate(bank):
                    m = np.ones((128, 128), np.float32) if kt < t \
                        else tri.astype(np.float32)
                    blk[:, 128 * s:128 * (s + 1)] = (
                        m * keymask[128 * kt:128 * (kt + 1)][:, None])
                maskb_arr[(b * NBANKS + bi) * 128:
                          (b * NBANKS + bi + 1) * 128] = blk
        maskb_arr = maskb_arr.astype(bf)

    in_maps = []
    for c in range(NCORES):
        rows = slice(128 * c, 128 * (c + 1))
        wk_c = np.ascontiguousarray(Wk_w[rows].T).astype(bf)
        wq_c = np.ascontiguousarray(Wq_w[rows].T).astype(bf)
        wv_c = np.ascontiguousarray(Wv_w[rows].T)  # [1024, 128] f32
        wv_pad = np.zeros((D, 130), np.float32)
        wv_pad[:, 0:64] = wv_c[:, 0:64]
        wv_pad[:, 65:129] = wv_c[:, 64:128]
        bv_pad = np.zeros((1, 130), np.float32)
        bv_pad[0, 0:64] = Wv_b[128 * c:128 * c + 64]
        bv_pad[0, 64] = 1.0
        bv_pad[0, 65:129] = Wv_b[128 * c + 64:128 * c + 128]
        bv_pad[0, 129] = 1.0
        im = {
            "xk_t": xk, "xq_t": xq, "xv_t": xv,
            "wk_h": wk_c, "wq_h": wq_c, "wv_p": wv_pad.astype(bf),
            "wo_r": wo_r,
            "bk_h": np.ascontiguousarray(
                Wk_b[rows].reshape(128, 1)).astype(np.float32),
            "bq_h": np.ascontiguousarray(
                Wq_b[rows].reshape(128, 1)).astype(np.float32),
            "bv_p": bv_pad, "bo_r": bo,
            "trimask": tri.astype(bf),
        }
        if general:
            im["maskb"] = maskb_arr
        in_maps.append(im)

    _BUILT["last_maps"] = in_maps
    res = run_bass_kernel_spmd(nc, in_maps, core_ids=list(range(NCORES)))
    _BUILT["last_result"] = res
    _BUILT["nc"] = nc

    outf = np.empty((B, S, D), np.float32)
    for c in range(NCORES):
        b, T = c // 4, c % 4
        outf[b, 512 * T:512 * (T + 1)] = res.results[c]["out"]
    return outf
